# revision 13
# baseline (speedup 1.0000x reference)
"""7x7 'same' 2D convolution over [128, 512, 512] f32, data-parallel on 8 NeuronCores.

Banded-Toeplitz formulation on the TensorEngine with 64x64 array
packing: the PE array runs as 4 independent 64x64 tiles
(tile_position=(64r, 64g)), each computing a 58-row output block
    out[i0+m, j] = sum_v sum_{r'} T_v[r', m] * xpad[i0+r', j+v]
with T_v[r', m] = w[r'-m, v] (band, r'<64, m<58). The 7 column taps (v)
accumulate into PSUM; tile (s, r2, g2) covers out rows
232s + 116g2 + 58r2 + [0,58) and writes PSUM bank (s, r2), partitions
[64g2, 64g2+58). One 4-tile tap sweep streams in one N=512 matmul
time (the TensorE issues an LDWEIGHTS+MATMUL pair every ~34ns, so
4 pairs per 237ns sweep leaves issue headroom; finer 32x32 packing
is issue-bound and slower). 8 tiles cover rows 0..463 of an image;
rows 464..511 of four consecutive images batch into one 4-tile
"tail" group.

Inputs are cast to fp16 and pre-staged host-side into the SBUF slab
layout (partition 64r+p, slab q, col) = padded row 116q + 58r + p
(slab 4 = tail rows, duplicated on both strips). All images' loads
are prefetched up front on the sync ring (two DMAs per image so the
s=0 matmuls depend only on the first). Accumulation is fp32; outputs
are stored as raw bf16 PSUM-bank dumps (vector/scalar copies, stores
rotating the gpsimd/sync/scalar rings) and un-permuted on the host.
"""

import numpy as np

B, H, W = 128, 512, 512
KS = 7
PAD = (KS - 1) // 2          # 3
HP = H + 2 * PAD             # 518
N_CORES = 8
PER_CORE = B // N_CORES      # 16
TS = 58                      # output rows per 64x64 tile (64 - 6)
NS = 5                       # slabs per image (4 group-1 + 1 tail)
TAILM = H - 8 * TS           # 48 tail output rows per image
TAILK = TAILM + KS - 1       # 54


def _build_program():
    import concourse.bass as bass
    import concourse.tile as tile
    from concourse import bacc, mybir

    f16 = mybir.dt.float16
    bf16 = mybir.dt.bfloat16
    f32 = mybir.dt.float32

    nc = bacc.Bacc("TRN2", target_bir_lowering=False, debug=False,
                   num_devices=N_CORES)
    x_ext = nc.declare_dram_parameter("x", [PER_CORE, 128, NS * HP], f16,
                                      isOutput=False)
    t_ext = nc.declare_dram_parameter("toep", [128, KS * TS], f16,
                                      isOutput=False)
    # og[img, s, r] = dump of PSUM bank (s, r):
    #   row 64g+p  ->  out row 232s + 116g + 58r + p   (valid p < 58)
    og_ext = nc.declare_dram_parameter("og", [PER_CORE, 2, 2, 128, W],
                                       bf16, isOutput=True)
    # ot[tg, r] = tail bank dump: row 64g+p -> img 4tg + 2g + r,
    #   out row 464 + p  (valid p < 48)
    ot_ext = nc.declare_dram_parameter("ot", [PER_CORE // 4, 2, 128, W],
                                       bf16, isOutput=True)

    with tile.TileContext(nc) as tc:
        with (
            tc.tile_pool(name="toep", bufs=1) as toep_pool,
            tc.tile_pool(name="xa", bufs=PER_CORE) as xa_pool,
            tc.tile_pool(name="xb", bufs=PER_CORE) as xb_pool,
            tc.tile_pool(name="psum", bufs=8, space="PSUM") as psum_pool,
            tc.tile_pool(name="outs", bufs=12) as out_pool,
        ):
            toep_sb = toep_pool.tile([128, KS * TS], f16)
            nc.sync.dma_start(out=toep_sb[:], in_=t_ext[:])

            # loads go on the dedicated sync ring, prefetched a bounded
            # number of images ahead (deep prefetch makes the tile
            # scheduler's ring-credit waits block the sync engine)
            sta, stb = {}, {}

            def load(img):
                if img >= PER_CORE:
                    return
                st_a = xa_pool.tile([128, 2 * HP], f16, name=f"sta{img}",
                                    tag="sta")
                nc.sync.dma_start(out=st_a[:], in_=x_ext[img, :, :2 * HP])
                st_b = xb_pool.tile([128, 3 * HP], f16, name=f"stb{img}",
                                    tag="stb")
                nc.sync.dma_start(out=st_b[:], in_=x_ext[img, :, 2 * HP:])
                sta[img], stb[img] = st_a, st_b

            for img in range(3):
                load(img)

            def cyc(seq):
                while True:
                    for e in seq:
                        yield e

            copy_engines = cyc([nc.vector, nc.vector, nc.scalar])
            store_rings = cyc([nc.gpsimd, nc.scalar, nc.gpsimd])

            def evac(ps, dst):
                o_sb = out_pool.tile([128, W], bf16, name="o", tag="osb")
                eng = next(copy_engines)
                if eng is nc.scalar:
                    eng.copy(o_sb[:], ps[:])
                else:
                    eng.tensor_copy(o_sb[:], ps[:])
                next(store_rings).dma_start(out=dst, in_=o_sb[:])

            for img in range(PER_CORE):
                load(img + 3)
                # s-groups sequential: only 2 PSUM banks live per group,
                # so allocation never stalls on evacuation of 4 banks.
                for s in range(2):
                    stage = sta[img] if s == 0 else stb[img]
                    ps = [psum_pool.tile([128, W], f32, name=f"ps{r}",
                                         tag="acc") for r in range(2)]
                    for v in range(KS):
                        for g in range(2):
                            for r in range(2):
                                nc.tensor.matmul(
                                    ps[r][64 * g:64 * g + TS, :],
                                    toep_sb[64 * r:64 * r + 64,
                                            TS * v:TS * (v + 1)],
                                    stage[64 * r:64 * r + 64,
                                          g * HP + v:g * HP + v + W],
                                    start=(v == 0),
                                    stop=(v == KS - 1),
                                    tile_position=(64 * r, 64 * g),
                                )
                    for r in range(2):
                        evac(ps[r], og_ext[img, s, r])

                    if s == 0 and img % 4 == 3:
                        # tail group between the two s-groups so its
                        # evacuations interleave with s=1 compute
                        tg = img // 4
                        pst = [psum_pool.tile([128, W], f32,
                                              name=f"pt{r}", tag="acc")
                               for r in range(2)]
                        for v in range(KS):
                            for j in range(4):
                                r, g = j % 2, j // 2
                                nc.tensor.matmul(
                                    pst[r][64 * g:64 * g + TAILM, :],
                                    toep_sb[64 * r:64 * r + TAILK,
                                            TS * v:TS * v + TAILM],
                                    stb[4 * tg + j][
                                        64 * r:64 * r + TAILK,
                                        2 * HP + v:2 * HP + v + W],
                                    start=(v == 0),
                                    stop=(v == KS - 1),
                                    tile_position=(64 * r, 64 * g),
                                )
                        for r in range(2):
                            evac(pst[r], ot_ext[tg, r])
    nc.finalize()
    return nc


def _host_prep(x, w):
    x = np.asarray(x, dtype=np.float32)
    w = np.asarray(w, dtype=np.float32)
    # padded images with extra zero rows (slab-4 strip-1 reads to 585)
    xpad = np.zeros((B, 586, HP), dtype=np.float16)
    xpad[:, PAD:PAD + H, PAD:PAD + W] = x
    # slab layout: (p, q) -> padded row 116q + 58*(p//64) + p%64;
    # slab 4 = tail rows 464+, duplicated on both 64-row strips
    p = np.arange(128)
    q = np.arange(NS)
    ridx = 116 * q[None, :] + 58 * (p[:, None] // 64) + (p[:, None] % 64)
    ridx[:, 4] = 464 + (p % 64)
    xslab = np.ascontiguousarray(
        xpad[:, ridx, :].reshape(B, 128, NS * HP))
    # Toeplitz band [64, 58] per tap, replicated on both partition strips
    toep = np.zeros((128, KS * TS), dtype=np.float16)
    w16 = w.astype(np.float16)
    idx = np.arange(TS)
    for st in range(2):
        for v in range(KS):
            for d in range(KS):
                toep[64 * st + idx + d, TS * v + idx] = w16[d, v]
    return xslab, toep


def _execute(x, w, **run_kwargs):
    from concourse.bass_utils import run_bass_kernel_spmd

    xslab, toep = _host_prep(x, w)
    nc = _build_program()
    in_maps = [
        {"x": xslab[c * PER_CORE:(c + 1) * PER_CORE], "toep": toep}
        for c in range(N_CORES)
    ]
    last_err = None
    for _attempt in range(3):
        try:
            res = run_bass_kernel_spmd(nc, in_maps,
                                       core_ids=list(range(N_CORES)),
                                       **run_kwargs)
            break
        except Exception as e:  # transient NRT execute flakes -> retry
            last_err = e
    else:
        raise last_err
    out = np.empty((B, H, W), dtype=np.float32)
    for c in range(N_CORES):
        sl = slice(c * PER_CORE, (c + 1) * PER_CORE)
        og = np.asarray(res.results[c]["og"], dtype=np.float32)
        ot = np.asarray(res.results[c]["ot"], dtype=np.float32)
        og6 = og.reshape(PER_CORE, 2, 2, 2, 64, W)[:, :, :, :, :TS, :]
        # [img, s, r, g, p, w] -> row = 232s + 116g + 58r + p
        out[sl, :8 * TS, :] = og6.transpose(0, 1, 3, 2, 4, 5).reshape(
            PER_CORE, 8 * TS, W)
        ot5 = ot.reshape(PER_CORE // 4, 2, 2, 64, W)[:, :, :, :TAILM, :]
        # [tg, r, g, p, w] -> img 4tg + 2g + r, row 464 + p
        out[sl, 8 * TS:, :] = ot5.transpose(0, 2, 1, 3, 4).reshape(
            PER_CORE, TAILM, W)
    return out, res


def kernel(x, w):
    out, _ = _execute(x, w)
    return out


# revision 16
# speedup vs baseline: 1.1830x; 1.1830x over previous
"""7x7 'same' 2D convolution over [128, 512, 512] f32, data-parallel on 8 NeuronCores.

Banded-Toeplitz formulation on the TensorEngine with 64x64 array
packing: the PE array runs as 4 independent 64x64 tiles
(tile_position=(64r, 64g)), each computing a 58-row output block
    out[i0+m, j] = sum_v sum_{r'} T_v[r', m] * xpad[i0+r', j+v]
with T_v[r', m] = w[r'-m, v] (band, r'<64, m<58). The 7 column taps (v)
accumulate into PSUM; tile (s, r2, g2) covers out rows
232s + 116g2 + 58r2 + [0,58) and writes PSUM bank (s, r2), partitions
[64g2, 64g2+58). One 4-tile tap sweep streams in one N=512 matmul
time (the TensorE issues an LDWEIGHTS+MATMUL pair every ~34ns, so 4
pairs per 237ns sweep leaves issue headroom; finer 32x32 packing is
issue-bound and slower). 8 tiles cover rows 0..463 of an image; rows
464..511 of four consecutive images batch into one 4-tile "tail"
group run between the owning image's two s-groups.

Inputs are cast to fp16 and pre-staged host-side into the SBUF slab
layout (partition 64r+p, slab q, col) = padded row 116q + 58r + p
(slab 4 = tail rows, duplicated on both strips), one 663KB DMA per
image on the sync ring, prefetched 3 images ahead. DMA count is kept
low (16 loads + 20 stores) because the tile framework recycles a
small pool of DMA-completion semaphores and the reuse credit waits
block whichever in-order engine issues the trigger. PSUM banks are
evacuated f32->bf16 (vector/scalar alternating) into one [128, 4*512]
SBUF tile per image, stored with a single DMA on the gpsimd ring;
the bank-dump layout is un-permuted on the host.
"""

import numpy as np

B, H, W = 128, 512, 512
KS = 7
PAD = (KS - 1) // 2          # 3
HP = H + 2 * PAD             # 518
N_CORES = 8
PER_CORE = B // N_CORES      # 16
TS = 58                      # output rows per 64x64 tile (64 - 6)
NS = 5                       # slabs per image (4 group-1 + 1 tail)
TAILM = H - 8 * TS           # 48 tail output rows per image
TAILK = TAILM + KS - 1       # 54


def _build_program():
    import concourse.bass as bass
    import concourse.tile as tile
    from concourse import bacc, mybir

    f16 = mybir.dt.float16
    bf16 = mybir.dt.bfloat16
    f32 = mybir.dt.float32

    nc = bacc.Bacc("TRN2", target_bir_lowering=False, debug=False,
                   num_devices=N_CORES)
    x_ext = nc.declare_dram_parameter("x", [PER_CORE, 128, NS * HP], f16,
                                      isOutput=False)
    t_ext = nc.declare_dram_parameter("toep", [128, KS * TS], f16,
                                      isOutput=False)
    # og[img, p, 2s+r, c]: bank (s, r) dump in column block 2s+r;
    #   partition 64g+p -> out row 232s + 116g + 58r + p  (valid p < 58)
    og_ext = nc.declare_dram_parameter("og", [PER_CORE, 128, 4 * W],
                                       bf16, isOutput=True)
    # ot[tg, p, r, c]: tail banks; partition 64g+p -> img 4tg + 2g + r,
    #   out row 464 + p  (valid p < 48)
    ot_ext = nc.declare_dram_parameter("ot", [PER_CORE // 4, 128, 2 * W],
                                       bf16, isOutput=True)

    with tile.TileContext(nc) as tc:
        with (
            tc.tile_pool(name="toep", bufs=1) as toep_pool,
            tc.tile_pool(name="xs", bufs=PER_CORE) as xs_pool,
            tc.tile_pool(name="psum", bufs=8, space="PSUM") as psum_pool,
            tc.tile_pool(name="outs", bufs=5) as out_pool,
            tc.tile_pool(name="touts", bufs=2) as tail_pool,
        ):
            toep_sb = toep_pool.tile([128, KS * TS], f16)
            nc.sync.dma_start(out=toep_sb[:], in_=t_ext[:])

            stages = {}

            def load(img):
                if img >= PER_CORE:
                    return
                st = xs_pool.tile([128, NS * HP], f16, name=f"st{img}",
                                  tag="st")
                nc.sync.dma_start(out=st[:], in_=x_ext[img])
                stages[img] = st

            for img in range(3):
                load(img)

            def cyc(seq):
                while True:
                    for e in seq:
                        yield e

            copy_engines = cyc([nc.vector, nc.scalar])

            def evac(ps, o_sb, blk):
                eng = next(copy_engines)
                dst = o_sb[:, blk * W:(blk + 1) * W]
                if eng is nc.scalar:
                    eng.copy(dst, ps[:])
                else:
                    eng.tensor_copy(dst, ps[:])

            for img in range(PER_CORE):
                load(img + 3)
                o_sb = out_pool.tile([128, 4 * W], bf16, name="o",
                                     tag="osb")
                # s-groups sequential: only 2 PSUM banks live per group,
                # so allocation never stalls on evacuation of 4 banks.
                for s in range(2):
                    stage = stages[img]
                    ps = [psum_pool.tile([128, W], f32, name=f"ps{r}",
                                         tag="acc") for r in range(2)]
                    for v in range(KS):
                        for g in range(2):
                            for r in range(2):
                                nc.tensor.matmul(
                                    ps[r][64 * g:64 * g + TS, :],
                                    toep_sb[64 * r:64 * r + 64,
                                            TS * v:TS * (v + 1)],
                                    stage[64 * r:64 * r + 64,
                                          (2 * s + g) * HP + v:
                                          (2 * s + g) * HP + v + W],
                                    start=(v == 0),
                                    stop=(v == KS - 1),
                                    tile_position=(64 * r, 64 * g),
                                )
                    for r in range(2):
                        evac(ps[r], o_sb, 2 * s + r)

                    if s == 0 and img % 4 == 3:
                        # tail group between the two s-groups so its
                        # evacuations interleave with s=1 compute
                        tg = img // 4
                        ot_sb = tail_pool.tile([128, 2 * W], bf16,
                                               name="ot", tag="otsb")
                        pst = [psum_pool.tile([128, W], f32,
                                              name=f"pt{r}", tag="acc")
                               for r in range(2)]
                        for v in range(KS):
                            for j in range(4):
                                r, g = j % 2, j // 2
                                nc.tensor.matmul(
                                    pst[r][64 * g:64 * g + TAILM, :],
                                    toep_sb[64 * r:64 * r + TAILK,
                                            TS * v:TS * v + TAILM],
                                    stages[4 * tg + j][
                                        64 * r:64 * r + TAILK,
                                        4 * HP + v:4 * HP + v + W],
                                    start=(v == 0),
                                    stop=(v == KS - 1),
                                    tile_position=(64 * r, 64 * g),
                                )
                        for r in range(2):
                            evac(pst[r], ot_sb, r)
                        nc.gpsimd.dma_start(out=ot_ext[tg], in_=ot_sb[:])
                nc.gpsimd.dma_start(out=og_ext[img], in_=o_sb[:])
    nc.finalize()
    return nc


def _host_prep(x, w):
    x = np.asarray(x, dtype=np.float32)
    w = np.asarray(w, dtype=np.float32)
    # padded images with extra zero rows (slab-4 strip-1 reads to 585)
    xpad = np.zeros((B, 586, HP), dtype=np.float16)
    xpad[:, PAD:PAD + H, PAD:PAD + W] = x
    # slab layout: (p, q) -> padded row 116q + 58*(p//64) + p%64;
    # slab 4 = tail rows 464+, duplicated on both 64-row strips
    p = np.arange(128)
    q = np.arange(NS)
    ridx = 116 * q[None, :] + 58 * (p[:, None] // 64) + (p[:, None] % 64)
    ridx[:, 4] = 464 + (p % 64)
    xslab = np.ascontiguousarray(
        xpad[:, ridx, :].reshape(B, 128, NS * HP))
    # Toeplitz band [64, 58] per tap, replicated on both partition strips
    toep = np.zeros((128, KS * TS), dtype=np.float16)
    w16 = w.astype(np.float16)
    idx = np.arange(TS)
    for st in range(2):
        for v in range(KS):
            for d in range(KS):
                toep[64 * st + idx + d, TS * v + idx] = w16[d, v]
    return xslab, toep


def _execute(x, w, **run_kwargs):
    from concourse.bass_utils import run_bass_kernel_spmd

    xslab, toep = _host_prep(x, w)
    nc = _build_program()
    in_maps = [
        {"x": xslab[c * PER_CORE:(c + 1) * PER_CORE], "toep": toep}
        for c in range(N_CORES)
    ]
    last_err = None
    for _attempt in range(3):
        try:
            res = run_bass_kernel_spmd(nc, in_maps,
                                       core_ids=list(range(N_CORES)),
                                       **run_kwargs)
            break
        except Exception as e:  # transient NRT execute flakes -> retry
            last_err = e
    else:
        raise last_err
    out = np.empty((B, H, W), dtype=np.float32)
    for c in range(N_CORES):
        sl = slice(c * PER_CORE, (c + 1) * PER_CORE)
        og = np.asarray(res.results[c]["og"], dtype=np.float32)
        ot = np.asarray(res.results[c]["ot"], dtype=np.float32)
        # og [img, 64g+p, 2s+r, c] -> row 232s + 116g + 58r + p
        og6 = og.reshape(PER_CORE, 2, 64, 2, 2, W)[:, :, :TS]
        # [img, g, p, s, r, c] -> [img, s, g, r, p, c]
        out[sl, :8 * TS, :] = og6.transpose(0, 3, 1, 4, 2, 5).reshape(
            PER_CORE, 8 * TS, W)
        # ot [tg, 64g+p, r, c] -> img 4tg + 2g + r, row 464 + p
        ot5 = ot.reshape(PER_CORE // 4, 2, 64, 2, W)[:, :, :TAILM]
        # [tg, g, p, r, c] -> [tg, g, r, p, c]
        out[sl, 8 * TS:, :] = ot5.transpose(0, 1, 3, 2, 4).reshape(
            PER_CORE, TAILM, W)
    return out, res


def kernel(x, w):
    out, _ = _execute(x, w)
    return out


# revision 21
# speedup vs baseline: 1.1963x; 1.0112x over previous
"""7x7 'same' 2D convolution over [128, 512, 512] f32, data-parallel on 8 NeuronCores.

Banded-Toeplitz formulation on the TensorEngine with 64x64 array
packing: the PE array runs as 4 independent 64x64 tiles
(tile_position=(64r, 64g)), each computing a 58-row output block
    out[i0+m, j] = sum_v sum_{r'} T_v[r', m] * xpad[i0+r', j+v]
with T_v[r', m] = w[r'-m, v] (band, r'<64, m<58). The 7 column taps (v)
accumulate into PSUM; tile (s, r2, g2) covers out rows
232s + 116g2 + 58r2 + [0,58) and writes PSUM bank (s, r2), partitions
[64g2, 64g2+58). One 4-tile tap sweep streams in one N=512 matmul
time (the TensorE issues an LDWEIGHTS+MATMUL pair every ~34ns, so 4
pairs per 237ns sweep leaves issue headroom; finer 32x32 packing is
issue-bound and slower). 8 tiles cover rows 0..463 of an image; rows
464..511 of four consecutive images batch into one 4-tile "tail"
group run between image 4t+2's two s-groups (one slot before the
last member image, so the final image's slot drains quickly).

Inputs are cast to fp16 and pre-staged host-side into the SBUF slab
layout (partition 64r+p, slab q, col) = padded row 116q + 58r + p
(slab 4 = tail rows, duplicated on both strips), one 663KB DMA per
image on the sync ring, prefetched 3 images ahead. DMA count is kept
low (16 loads + 20 stores) because the tile framework recycles a
small pool of DMA-completion semaphores and the reuse credit waits
block whichever in-order engine issues the trigger. PSUM banks are
evacuated f32->bf16 (vector/scalar alternating) into one [128, 4*512]
SBUF tile per image, stored per s-group half on the gpsimd ring;
the bank-dump layout is un-permuted on the host.
"""

import numpy as np

B, H, W = 128, 512, 512
KS = 7
PAD = (KS - 1) // 2          # 3
HP = H + 2 * PAD             # 518
N_CORES = 8
PER_CORE = B // N_CORES      # 16
TS = 58                      # output rows per 64x64 tile (64 - 6)
NS = 5                       # slabs per image (4 group-1 + 1 tail)
TAILM = H - 8 * TS           # 48 tail output rows per image
TAILK = TAILM + KS - 1       # 54


def _build_program():
    import concourse.bass as bass
    import concourse.tile as tile
    from concourse import bacc, mybir

    f16 = mybir.dt.float16
    bf16 = mybir.dt.bfloat16
    f32 = mybir.dt.float32

    nc = bacc.Bacc("TRN2", target_bir_lowering=False, debug=False,
                   num_devices=N_CORES)
    x_ext = nc.declare_dram_parameter("x", [PER_CORE, 128, NS * HP], f16,
                                      isOutput=False)
    t_ext = nc.declare_dram_parameter("toep", [128, KS * TS], f16,
                                      isOutput=False)
    # og[img, p, 2s+r, c]: bank (s, r) dump in column block 2s+r;
    #   partition 64g+p -> out row 232s + 116g + 58r + p  (valid p < 58)
    og_ext = nc.declare_dram_parameter("og", [PER_CORE, 128, 4 * W],
                                       bf16, isOutput=True)
    # ot[tg, p, r, c]: tail banks; partition 64g+p -> img 4tg + 2g + r,
    #   out row 464 + p  (valid p < 48)
    ot_ext = nc.declare_dram_parameter("ot", [PER_CORE // 4, 128, 2 * W],
                                       bf16, isOutput=True)

    with tile.TileContext(nc) as tc:
        with (
            tc.tile_pool(name="toep", bufs=1) as toep_pool,
            tc.tile_pool(name="xs", bufs=PER_CORE) as xs_pool,
            tc.tile_pool(name="psum", bufs=8, space="PSUM") as psum_pool,
            tc.tile_pool(name="outs", bufs=5) as out_pool,
            tc.tile_pool(name="touts", bufs=2) as tail_pool,
        ):
            toep_sb = toep_pool.tile([128, KS * TS], f16)
            nc.sync.dma_start(out=toep_sb[:], in_=t_ext[:])

            stages = {}

            def load(img):
                if img >= PER_CORE:
                    return
                st = xs_pool.tile([128, NS * HP], f16, name=f"st{img}",
                                  tag="st")
                if img < 2:
                    # split the first loads so compute starts sooner
                    nc.sync.dma_start(out=st[:, :2 * HP],
                                      in_=x_ext[img, :, :2 * HP])
                    nc.sync.dma_start(out=st[:, 2 * HP:],
                                      in_=x_ext[img, :, 2 * HP:])
                else:
                    nc.sync.dma_start(out=st[:], in_=x_ext[img])
                stages[img] = st

            for img in range(3):
                load(img)

            def cyc(seq):
                while True:
                    for e in seq:
                        yield e

            copy_engines = cyc([nc.vector, nc.scalar])

            def evac(ps, o_sb, blk):
                eng = next(copy_engines)
                dst = o_sb[:, blk * W:(blk + 1) * W]
                if eng is nc.scalar:
                    eng.copy(dst, ps[:])
                else:
                    eng.tensor_copy(dst, ps[:])

            for img in range(PER_CORE):
                load(img + 3)
                o_sb = out_pool.tile([128, 4 * W], bf16, name="o",
                                     tag="osb")
                # s-groups sequential: only 2 PSUM banks live per group,
                # so allocation never stalls on evacuation of 4 banks.
                for s in range(2):
                    stage = stages[img]
                    ps = [psum_pool.tile([128, W], f32, name=f"ps{r}",
                                         tag="acc") for r in range(2)]
                    for v in range(KS):
                        for g in range(2):
                            for r in range(2):
                                nc.tensor.matmul(
                                    ps[r][64 * g:64 * g + TS, :],
                                    toep_sb[64 * r:64 * r + 64,
                                            TS * v:TS * (v + 1)],
                                    stage[64 * r:64 * r + 64,
                                          (2 * s + g) * HP + v:
                                          (2 * s + g) * HP + v + W],
                                    start=(v == 0),
                                    stop=(v == KS - 1),
                                    tile_position=(64 * r, 64 * g),
                                )
                    for r in range(2):
                        evac(ps[r], o_sb, 2 * s + r)
                    # store each s-group half as soon as its copies land
                    nc.gpsimd.dma_start(
                        out=og_ext[img, :, 2 * s * W:2 * (s + 1) * W],
                        in_=o_sb[:, 2 * s * W:2 * (s + 1) * W])

                    if s == 0 and img % 4 == 2:
                        # tail group between the two s-groups so its
                        # evacuations interleave with s=1 compute
                        tg = img // 4
                        ot_sb = tail_pool.tile([128, 2 * W], bf16,
                                               name="ot", tag="otsb")
                        pst = [psum_pool.tile([128, W], f32,
                                              name=f"pt{r}", tag="acc")
                               for r in range(2)]
                        for v in range(KS):
                            for j in range(4):
                                r, g = j % 2, j // 2
                                nc.tensor.matmul(
                                    pst[r][64 * g:64 * g + TAILM, :],
                                    toep_sb[64 * r:64 * r + TAILK,
                                            TS * v:TS * v + TAILM],
                                    stages[4 * tg + j][
                                        64 * r:64 * r + TAILK,
                                        4 * HP + v:4 * HP + v + W],
                                    start=(v == 0),
                                    stop=(v == KS - 1),
                                    tile_position=(64 * r, 64 * g),
                                )
                        for r in range(2):
                            evac(pst[r], ot_sb, r)
                        nc.gpsimd.dma_start(out=ot_ext[tg], in_=ot_sb[:])
    nc.finalize()
    return nc


def _host_prep(x, w):
    x = np.asarray(x, dtype=np.float32)
    w = np.asarray(w, dtype=np.float32)
    # padded images with extra zero rows (slab-4 strip-1 reads to 585)
    xpad = np.zeros((B, 586, HP), dtype=np.float16)
    xpad[:, PAD:PAD + H, PAD:PAD + W] = x
    # slab layout: (p, q) -> padded row 116q + 58*(p//64) + p%64;
    # slab 4 = tail rows 464+, duplicated on both 64-row strips
    p = np.arange(128)
    q = np.arange(NS)
    ridx = 116 * q[None, :] + 58 * (p[:, None] // 64) + (p[:, None] % 64)
    ridx[:, 4] = 464 + (p % 64)
    xslab = np.ascontiguousarray(
        xpad[:, ridx, :].reshape(B, 128, NS * HP))
    # Toeplitz band [64, 58] per tap, replicated on both partition strips
    toep = np.zeros((128, KS * TS), dtype=np.float16)
    w16 = w.astype(np.float16)
    idx = np.arange(TS)
    for st in range(2):
        for v in range(KS):
            for d in range(KS):
                toep[64 * st + idx + d, TS * v + idx] = w16[d, v]
    return xslab, toep


def _execute(x, w, **run_kwargs):
    from concourse.bass_utils import run_bass_kernel_spmd

    xslab, toep = _host_prep(x, w)
    nc = _build_program()
    in_maps = [
        {"x": xslab[c * PER_CORE:(c + 1) * PER_CORE], "toep": toep}
        for c in range(N_CORES)
    ]
    last_err = None
    for _attempt in range(3):
        try:
            res = run_bass_kernel_spmd(nc, in_maps,
                                       core_ids=list(range(N_CORES)),
                                       **run_kwargs)
            break
        except Exception as e:  # transient NRT execute flakes -> retry
            last_err = e
    else:
        raise last_err
    out = np.empty((B, H, W), dtype=np.float32)
    for c in range(N_CORES):
        sl = slice(c * PER_CORE, (c + 1) * PER_CORE)
        og = np.asarray(res.results[c]["og"], dtype=np.float32)
        ot = np.asarray(res.results[c]["ot"], dtype=np.float32)
        # og [img, 64g+p, 2s+r, c] -> row 232s + 116g + 58r + p
        og6 = og.reshape(PER_CORE, 2, 64, 2, 2, W)[:, :, :TS]
        # [img, g, p, s, r, c] -> [img, s, g, r, p, c]
        out[sl, :8 * TS, :] = og6.transpose(0, 3, 1, 4, 2, 5).reshape(
            PER_CORE, 8 * TS, W)
        # ot [tg, 64g+p, r, c] -> img 4tg + 2g + r, row 464 + p
        ot5 = ot.reshape(PER_CORE // 4, 2, 64, 2, W)[:, :, :TAILM]
        # [tg, g, p, r, c] -> [tg, g, r, p, c]
        out[sl, 8 * TS:, :] = ot5.transpose(0, 1, 3, 2, 4).reshape(
            PER_CORE, TAILM, W)
    return out, res


def kernel(x, w):
    out, _ = _execute(x, w)
    return out
